# revision 23
# baseline (speedup 1.0000x reference)
"""Trainium2 Bass kernel for a 12-head self-attention block.

Reference computation (per batch b of 8):
    qkv = x @ w_qkv                      # (1024, 2304)
    q, k, v per head (12 heads, d=64)
    attn = softmax(q k^T / sqrt(64))
    ctx  = attn @ v                      # (1024, 768)
    y    = ctx @ w_proj + b_proj

Sharding: data parallel over the batch dim — batch b runs on core b.
Each core gets the full weights and its own x slice; no collectives.

Per-core dataflow:
  - Matmul operands are bf16 (fp32 moving operands halve the PE's
    SBUF stream rate; bf16 runs at 1 col/cycle).  All accumulation is
    fp32 in PSUM, softmax statistics stay fp32.
  - X^T built once via PE transposes (contraction dim must sit on
    partitions for the TensorE).
  - q^T/k^T tiles (heads packed two per 128-partition tile) come from
    qk^T = W_qk^T @ X^T so the S matmul needs no further transposes.
  - V is computed in natural (token, feature) layout with a column of
    ones appended per head: the attn @ v matmul then yields the softmax
    denominator in PSUM partition 64 for free.
  - S^T(keys, queries) per 128-key tile -> exp on ScalarE (softmax max
    subtraction is skipped: logits are ~N(0,1), exp is safe in fp32)
    -> O^T accumulated over key tiles in PSUM.
  - Normalize: fast reciprocal of the denominator row, gpsimd
    partition-broadcast, one multiply; results assemble ctx^T which
    feeds the projection as the stationary operand.  Bias is a K=1
    matmul with a ones row.
  - Matmul output chunks never cross a PSUM bank (512 fp32) boundary.
"""

import numpy as np

N = 1024          # tokens per batch (32*32)
C = 768           # model dim
NH = 12           # heads
D = 64            # head dim
NT = N // 128     # 8 token tiles
KC = C // 128     # 6 contraction tiles
SCALE = D ** -0.5
NCORES = 8

_CACHE = {}


def _build_nc():
    import concourse.bass as bass
    import concourse.tile as tile
    from concourse import bacc, mybir
    from concourse.masks import make_identity

    F32 = mybir.dt.float32
    BF16 = mybir.dt.bfloat16
    Exp = mybir.ActivationFunctionType.Exp

    nc = bacc.Bacc(None, target_bir_lowering=False)
    x = nc.declare_dram_parameter("x", [N, C], F32, isOutput=False)
    wqkv = nc.declare_dram_parameter("w_qkv", [C, 3 * C], F32, isOutput=False)
    wproj = nc.declare_dram_parameter("w_proj", [C, C], F32, isOutput=False)
    bproj = nc.declare_dram_parameter("b_proj", [1, C], F32, isOutput=False)
    y = nc.declare_dram_parameter("y", [N, C], F32, isOutput=True)

    with tile.TileContext(nc) as tc:
        from contextlib import ExitStack

        with ExitStack() as ctx:
            persist = ctx.enter_context(tc.tile_pool(name="persist", bufs=1))
            xT = persist.tile([128, KC, N], BF16)          # X^T (c, n)
            wqk = persist.tile([128, KC, 2 * C], BF16)     # W_q|W_k rows
            V = persist.tile([128, NT, NH, D + 2], BF16)   # v + ones col (+pad: 4B-aligned head stride)
            wp = persist.tile([128, KC, C], BF16)          # W_proj rows
            ctxT = persist.tile([128, KC, N], BF16)        # normalized ctx^T
            ident = persist.tile([128, 128], F32)
            ones_row = persist.tile([1, 128], BF16)
            ones_f32 = persist.tile([128, 128], F32)
            bias_sb = persist.tile([1, C], BF16)

            make_identity(nc, ident)
            nc.vector.memset(ones_f32[:], 1.0)
            nc.vector.tensor_copy(out=ones_row[:], in_=ones_f32[0:1, :])
            for _t in range(NT):
                # write ones in pairs (4-byte chunks): lone 2-byte strided
                # writes are not safe on the compute engines
                nc.any.tensor_copy(
                    out=V[:, _t, :, D:D + 2],
                    in_=ones_f32[:, 0:2 * NH].rearrange(
                        "p (h two) -> p h two", two=2
                    ),
                )

            psA = ctx.enter_context(
                tc.tile_pool(name="psA", bufs=2, space="PSUM")
            )
            psO = ctx.enter_context(
                tc.tile_pool(name="psO", bufs=2, space="PSUM")
            )

            # ---- Phase 0: load X, build X^T via PE transposes -------------
            with tc.tile_pool(name="xload", bufs=3) as xpool:
                for nt in range(NT):
                    xt_in = xpool.tile([128, C], F32, tag="x")
                    # split each row-tile load across two HWDGE queues
                    nc.sync.dma_start(
                        out=xt_in[:, 0:384],
                        in_=x[nt * 128:(nt + 1) * 128, 0:384],
                    )
                    nc.scalar.dma_start(
                        out=xt_in[:, 384:C],
                        in_=x[nt * 128:(nt + 1) * 128, 384:C],
                    )
                    ps = psA.tile([128, KC, 128], F32, tag="ps")
                    for kc in range(KC):
                        nc.tensor.transpose(
                            ps[:, kc, :],
                            xt_in[:, kc * 128:(kc + 1) * 128],
                            ident[:],
                        )
                    nc.vector.tensor_copy(
                        out=xT[:, :, nt * 128:(nt + 1) * 128], in_=ps[:]
                    )

            # ---- weight loads: SWDGE DMAs cast f32 -> bf16 in flight ----
            with tc.tile_pool(name="wv", bufs=1) as wvp:
                wv = wvp.tile([128, KC, C], BF16)
                for kc in range(KC):
                    nc.gpsimd.dma_start(
                        out=wv[:, kc, :],
                        in_=wqkv[kc * 128:(kc + 1) * 128, 2 * C:3 * C],
                    )
                for kc in range(KC):
                    nc.gpsimd.dma_start(
                        out=wqk[:, kc, :],
                        in_=wqkv[kc * 128:(kc + 1) * 128, 0:2 * C],
                    )
                for kc in range(KC):
                    nc.gpsimd.dma_start(
                        out=wp[:, kc, :],
                        in_=wproj[kc * 128:(kc + 1) * 128, :],
                    )
                nc.gpsimd.dma_start(out=bias_sb[:], in_=bproj[:])

                # ---- Phase 1: V = X @ W_v (natural layout) ----------------
                for t in range(NT):
                    ps = psA.tile([128, C], F32, tag="ps")
                    # chunks must not cross PSUM bank boundaries (512 f32)
                    for sl in (slice(0, 512), slice(512, C)):
                        for kc in range(KC):
                            nc.tensor.matmul(
                                ps[:, sl],
                                lhsT=xT[:, kc, t * 128:(t + 1) * 128],
                                rhs=wv[:, kc, sl],
                                start=(kc == 0),
                                stop=(kc == KC - 1),
                            )
                    nc.vector.tensor_copy(
                        out=V[:, t, :, 0:D],
                        in_=ps[:].rearrange("p (h d) -> p h d", h=NH),
                    )

            # ---- Phase 2: per head pair: q^T/k^T, then attention ----------
            qkpool = ctx.enter_context(tc.tile_pool(name="qk", bufs=2))
            ptpool = ctx.enter_context(tc.tile_pool(name="pt", bufs=6))
            bcpool = ctx.enter_context(tc.tile_pool(name="bc", bufs=3))
            oupool = ctx.enter_context(tc.tile_pool(name="ou", bufs=3))

            for j in range(NH // 2):   # head pairs (2j, 2j+1)
                qT = qkpool.tile([128, N], BF16, tag="qT")
                kT = qkpool.tile([128, N], BF16, tag="kT")
                # qk^T tile = W^T X^T for this pair's 128 output channels
                for dst, coff in ((qT, j * 128), (kT, C + j * 128)):
                    ps = psA.tile([128, N], F32, tag="ps")
                    for cch in range(2):
                        sl = slice(cch * 512, (cch + 1) * 512)
                        for kc in range(KC):
                            nc.tensor.matmul(
                                ps[:, sl],
                                lhsT=wqk[:, kc, coff:coff + 128],
                                rhs=xT[:, kc, sl],
                                start=(kc == 0),
                                stop=(kc == KC - 1),
                            )
                    nc.vector.tensor_copy(out=dst[:], in_=ps[:])

                for hh in range(2):
                    h = 2 * j + hh
                    pb = hh * 64
                    OT = psO.tile([D + 1, N], F32, tag="ot")
                    for t in range(NT):
                        S = psA.tile([128, N], F32, tag="ps")
                        for cch in range(2):
                            sl = slice(cch * 512, (cch + 1) * 512)
                            nc.tensor.matmul(
                                S[:, sl],
                                lhsT=kT[pb:pb + 64, t * 128:(t + 1) * 128],
                                rhs=qT[pb:pb + 64, sl],
                                start=True,
                                stop=True,
                            )
                        pT = ptpool.tile([128, N], BF16, tag="pt")
                        nc.scalar.activation(
                            out=pT[:], in_=S[:], func=Exp, scale=SCALE
                        )
                        for cch in range(2):
                            sl = slice(cch * 512, (cch + 1) * 512)
                            nc.tensor.matmul(
                                OT[:, sl],
                                lhsT=V[:, t, h, 0:D + 1],
                                rhs=pT[:, sl],
                                start=(t == 0),
                                stop=(t == NT - 1),
                            )
                    # Free the PSUM slots quickly: copy O^T and the
                    # denominator row to SBUF (DVE), then normalize from
                    # SBUF off the PE critical path.
                    # reciprocal_approx_fast is a bitwise custom-DVE op and
                    # must read from SBUF, not PSUM.
                    ou = oupool.tile([64, N], F32, tag="ou")
                    nc.vector.tensor_copy(out=ou[:], in_=OT[0:D, :])
                    den_sb = bcpool.tile([1, N], F32, tag="den")
                    nc.vector.tensor_copy(out=den_sb[:], in_=OT[D:D + 1, :])
                    bc = bcpool.tile([64, N], F32, tag="bc")
                    nc.vector.reciprocal_approx_fast(
                        out=bc[0:1, :], in_=den_sb[:]
                    )
                    nc.gpsimd.partition_broadcast(
                        bc[:], bc[0:1, :], channels=64
                    )
                    nc.vector.tensor_mul(
                        out=ctxT[pb:pb + 64, j, :], in0=ou[:], in1=bc[:]
                    )

            # ---- Phase 3: y = ctx @ W_proj + b ----------------------------
            outpool = ctx.enter_context(tc.tile_pool(name="out", bufs=3))
            for nt in range(NT):
                for cch in range(2):
                    sl = slice(cch * 384, (cch + 1) * 384)
                    ps = psA.tile([128, 384], F32, tag="ps",
                                  name=f"pj{nt}_{cch}")
                    for kc in range(KC):
                        nc.tensor.matmul(
                            ps[:],
                            lhsT=ctxT[:, kc, nt * 128:(nt + 1) * 128],
                            rhs=wp[:, kc, sl],
                            start=(kc == 0),
                            stop=False,
                        )
                    nc.tensor.matmul(
                        ps[:],
                        lhsT=ones_row[:],
                        rhs=bias_sb[:, sl],
                        start=False,
                        stop=True,
                    )
                    ob = outpool.tile([128, 384], F32, tag="ob")
                    nc.scalar.copy(ob[:], ps[:])
                    nc.sync.dma_start(
                        out=y[nt * 128:(nt + 1) * 128, sl], in_=ob[:]
                    )

    nc.finalize()
    return nc


def _get_nc():
    if "nc" not in _CACHE:
        _CACHE["nc"] = _build_nc()
    return _CACHE["nc"]


def _make_in_maps(x, w_qkv, w_proj, b_proj):
    B = x.shape[0]
    xb = np.ascontiguousarray(x.reshape(B, N, C).astype(np.float32))
    w_qkv = np.ascontiguousarray(w_qkv.astype(np.float32))
    w_proj = np.ascontiguousarray(w_proj.astype(np.float32))
    bp = np.ascontiguousarray(b_proj.reshape(1, C).astype(np.float32))
    return [
        {"x": xb[b], "w_qkv": w_qkv, "w_proj": w_proj, "b_proj": bp}
        for b in range(B)
    ]


def _run(in_maps, **kwargs):
    from concourse.bass_utils import run_bass_kernel_spmd

    nc = _get_nc()
    return run_bass_kernel_spmd(
        nc, in_maps, core_ids=list(range(NCORES)), **kwargs
    )


def kernel(x, w_qkv, w_proj, b_proj):
    B, H, W, _ = x.shape
    res = _run(_make_in_maps(x, w_qkv, w_proj, b_proj))
    out = np.stack([res.results[b]["y"] for b in range(B)])
    return out.reshape(B, H, W, C).astype(np.float32)


# revision 25
# speedup vs baseline: 1.0609x; 1.0609x over previous
"""Trainium2 Bass kernel for a 12-head self-attention block.

Reference computation (per batch b of 8):
    qkv = x @ w_qkv                      # (1024, 2304)
    q, k, v per head (12 heads, d=64)
    attn = softmax(q k^T / sqrt(64))
    ctx  = attn @ v                      # (1024, 768)
    y    = ctx @ w_proj + b_proj

Sharding: data parallel over the batch dim — batch b runs on core b.
Each core gets the full weights and its own x slice; no collectives.

Per-core dataflow:
  - Matmul operands are bf16 (fp32 moving operands halve the PE's
    SBUF stream rate; bf16 runs at 1 col/cycle).  All accumulation is
    fp32 in PSUM, softmax statistics stay fp32.
  - X^T built once via PE transposes (contraction dim must sit on
    partitions for the TensorE).
  - q^T/k^T tiles (heads packed two per 128-partition tile) come from
    qk^T = W_qk^T @ X^T so the S matmul needs no further transposes.
  - V is computed in natural (token, feature) layout with a column of
    ones appended per head: the attn @ v matmul then yields the softmax
    denominator in PSUM partition 64 for free.
  - S^T(keys, queries) per 128-key tile -> exp on ScalarE (softmax max
    subtraction is skipped: logits are ~N(0,1), exp is safe in fp32)
    -> O^T accumulated over key tiles in PSUM.
  - Normalize: fast reciprocal of the denominator row, gpsimd
    partition-broadcast, one multiply; results assemble ctx^T which
    feeds the projection as the stationary operand.  Bias is a K=1
    matmul with a ones row.
  - Matmul output chunks never cross a PSUM bank (512 fp32) boundary.
"""

import numpy as np

N = 1024          # tokens per batch (32*32)
C = 768           # model dim
NH = 12           # heads
D = 64            # head dim
NT = N // 128     # 8 token tiles
KC = C // 128     # 6 contraction tiles
SCALE = D ** -0.5
NCORES = 8

_CACHE = {}


def _build_nc():
    import concourse.bass as bass
    import concourse.tile as tile
    from concourse import bacc, mybir
    from concourse.masks import make_identity

    F32 = mybir.dt.float32
    BF16 = mybir.dt.bfloat16
    Exp = mybir.ActivationFunctionType.Exp

    nc = bacc.Bacc(None, target_bir_lowering=False)
    x = nc.declare_dram_parameter("x", [N, C], F32, isOutput=False)
    wqkv = nc.declare_dram_parameter("w_qkv", [C, 3 * C], F32, isOutput=False)
    wproj = nc.declare_dram_parameter("w_proj", [C, C], F32, isOutput=False)
    bproj = nc.declare_dram_parameter("b_proj", [1, C], F32, isOutput=False)
    y = nc.declare_dram_parameter("y", [N, C], F32, isOutput=True)

    with tile.TileContext(nc) as tc:
        from contextlib import ExitStack

        with ExitStack() as ctx:
            persist = ctx.enter_context(tc.tile_pool(name="persist", bufs=1))
            xT = persist.tile([128, KC, N], BF16)          # X^T (c, n)
            wqk = persist.tile([128, KC, 2 * C], BF16)     # W_q|W_k rows
            V = persist.tile([128, NT, NH, D + 2], BF16)   # v + ones col (+pad: 4B-aligned head stride)
            wp = persist.tile([128, KC, C], BF16)          # W_proj rows
            ctxT = persist.tile([128, KC, N], BF16)        # normalized ctx^T
            ident = persist.tile([128, 128], F32)
            ones_row = persist.tile([1, 128], BF16)
            ones_f32 = persist.tile([128, 128], F32)
            bias_sb = persist.tile([1, C], BF16)

            make_identity(nc, ident)
            nc.vector.memset(ones_f32[:], 1.0)
            nc.vector.tensor_copy(out=ones_row[:], in_=ones_f32[0:1, :])
            for _t in range(NT):
                # write ones in pairs (4-byte chunks): lone 2-byte strided
                # writes are not safe on the compute engines
                nc.any.tensor_copy(
                    out=V[:, _t, :, D:D + 2],
                    in_=ones_f32[:, 0:2 * NH].rearrange(
                        "p (h two) -> p h two", two=2
                    ),
                )

            psA = ctx.enter_context(
                tc.tile_pool(name="psA", bufs=2, space="PSUM")
            )
            psO = ctx.enter_context(
                tc.tile_pool(name="psO", bufs=2, space="PSUM")
            )

            # ---- Phase 0: load X, build X^T via PE transposes -------------
            with tc.tile_pool(name="xload", bufs=3) as xpool:
                for nt in range(NT):
                    xt_in = xpool.tile([128, C], F32, tag="x")
                    # split each row-tile load across two HWDGE queues
                    nc.sync.dma_start(
                        out=xt_in[:, 0:384],
                        in_=x[nt * 128:(nt + 1) * 128, 0:384],
                    )
                    nc.scalar.dma_start(
                        out=xt_in[:, 384:C],
                        in_=x[nt * 128:(nt + 1) * 128, 384:C],
                    )
                    ps = psA.tile([128, KC, 128], F32, tag="ps")
                    for kc in range(KC):
                        nc.tensor.transpose(
                            ps[:, kc, :],
                            xt_in[:, kc * 128:(kc + 1) * 128],
                            ident[:],
                        )
                    nc.vector.tensor_copy(
                        out=xT[:, :, nt * 128:(nt + 1) * 128], in_=ps[:]
                    )

            # ---- weight loads: SWDGE DMAs cast f32 -> bf16 in flight ----
            with tc.tile_pool(name="wv", bufs=1) as wvp:
                wv = wvp.tile([128, KC, C], BF16)
                for kc in range(KC):
                    nc.gpsimd.dma_start(
                        out=wv[:, kc, :],
                        in_=wqkv[kc * 128:(kc + 1) * 128, 2 * C:3 * C],
                    )
                for kc in range(KC):
                    nc.gpsimd.dma_start(
                        out=wqk[:, kc, :],
                        in_=wqkv[kc * 128:(kc + 1) * 128, 0:2 * C],
                    )
                for kc in range(KC):
                    nc.gpsimd.dma_start(
                        out=wp[:, kc, :],
                        in_=wproj[kc * 128:(kc + 1) * 128, :],
                    )
                nc.gpsimd.dma_start(out=bias_sb[:], in_=bproj[:])

                # ---- Phase 1: V = X @ W_v (natural layout) ----------------
                for t in range(NT):
                    ps = psA.tile([128, C], F32, tag="ps")
                    # chunks must not cross PSUM bank boundaries (512 f32)
                    for sl in (slice(0, 512), slice(512, C)):
                        for kc in range(KC):
                            nc.tensor.matmul(
                                ps[:, sl],
                                lhsT=xT[:, kc, t * 128:(t + 1) * 128],
                                rhs=wv[:, kc, sl],
                                start=(kc == 0),
                                stop=(kc == KC - 1),
                            )
                    nc.vector.tensor_copy(
                        out=V[:, t, :, 0:D],
                        in_=ps[:].rearrange("p (h d) -> p h d", h=NH),
                    )

            # ---- Phase 2: per head pair: q^T/k^T, then attention ----------
            qkpool = ctx.enter_context(tc.tile_pool(name="qk", bufs=2))
            ptpool = ctx.enter_context(tc.tile_pool(name="pt", bufs=6))
            bcpool = ctx.enter_context(tc.tile_pool(name="bc", bufs=3))
            oupool = ctx.enter_context(tc.tile_pool(name="ou", bufs=3))

            for j in range(NH // 2):   # head pairs (2j, 2j+1)
                qT = qkpool.tile([128, N], BF16, tag="qT")
                kT = qkpool.tile([128, N], BF16, tag="kT")
                # qk^T tile = W^T X^T for this pair's 128 output channels
                for dst, coff in ((qT, j * 128), (kT, C + j * 128)):
                    ps = psA.tile([128, N], F32, tag="ps")
                    for cch in range(2):
                        sl = slice(cch * 512, (cch + 1) * 512)
                        for kc in range(KC):
                            nc.tensor.matmul(
                                ps[:, sl],
                                lhsT=wqk[:, kc, coff:coff + 128],
                                rhs=xT[:, kc, sl],
                                start=(kc == 0),
                                stop=(kc == KC - 1),
                            )
                    nc.vector.tensor_copy(out=dst[:], in_=ps[:])

                for hh in range(2):
                    h = 2 * j + hh
                    pb = hh * 64
                    OT = psO.tile([D + 1, N], F32, tag="ot")
                    for t in range(NT):
                        S = psA.tile([128, N], F32, tag="ps")
                        for cch in range(2):
                            sl = slice(cch * 512, (cch + 1) * 512)
                            nc.tensor.matmul(
                                S[:, sl],
                                lhsT=kT[pb:pb + 64, t * 128:(t + 1) * 128],
                                rhs=qT[pb:pb + 64, sl],
                                start=True,
                                stop=True,
                            )
                        pT = ptpool.tile([128, N], BF16, tag="pt")
                        nc.scalar.activation(
                            out=pT[:], in_=S[:], func=Exp, scale=SCALE
                        )
                        for cch in range(2):
                            sl = slice(cch * 512, (cch + 1) * 512)
                            nc.tensor.matmul(
                                OT[:, sl],
                                lhsT=V[:, t, h, 0:D + 1],
                                rhs=pT[:, sl],
                                start=(t == 0),
                                stop=(t == NT - 1),
                            )
                    # Free the PSUM slots quickly: copy O^T and the
                    # denominator row to SBUF (DVE), then normalize from
                    # SBUF off the PE critical path.
                    # reciprocal_approx_fast is a bitwise custom-DVE op and
                    # must read from SBUF, not PSUM.
                    ou = oupool.tile([64, N], F32, tag="ou")
                    nc.vector.tensor_copy(out=ou[:], in_=OT[0:D, :])
                    den_sb = bcpool.tile([1, N], F32, tag="den")
                    nc.vector.tensor_copy(out=den_sb[:], in_=OT[D:D + 1, :])
                    bc = bcpool.tile([64, N], F32, tag="bc")
                    nc.vector.reciprocal_approx_fast(
                        out=bc[0:1, :], in_=den_sb[:]
                    )
                    nc.gpsimd.partition_broadcast(
                        bc[:], bc[0:1, :], channels=64
                    )
                    nc.vector.tensor_mul(
                        out=ctxT[pb:pb + 64, j, :], in0=ou[:], in1=bc[:]
                    )

            # ---- Phase 3: y = ctx @ W_proj + b ----------------------------
            outpool = ctx.enter_context(tc.tile_pool(name="out", bufs=3))
            for nt in range(NT):
                for cch in range(2):
                    sl = slice(cch * 384, (cch + 1) * 384)
                    ps = psA.tile([128, 384], F32, tag="ps",
                                  name=f"pj{nt}_{cch}")
                    for kc in range(KC):
                        nc.tensor.matmul(
                            ps[:],
                            lhsT=ctxT[:, kc, nt * 128:(nt + 1) * 128],
                            rhs=wp[:, kc, sl],
                            start=(kc == 0),
                            stop=False,
                        )
                    nc.tensor.matmul(
                        ps[:],
                        lhsT=ones_row[:],
                        rhs=bias_sb[:, sl],
                        start=False,
                        stop=True,
                    )
                    ob = outpool.tile([128, 384], F32, tag="ob")
                    nc.scalar.copy(ob[:], ps[:])
                    nc.sync.dma_start(
                        out=y[nt * 128:(nt + 1) * 128, sl], in_=ob[:]
                    )

    nc.finalize()
    return nc


def _get_nc():
    if "nc" not in _CACHE:
        _CACHE["nc"] = _build_nc()
    return _CACHE["nc"]


def _make_in_maps(x, w_qkv, w_proj, b_proj):
    B = x.shape[0]
    xb = np.ascontiguousarray(x.reshape(B, N, C).astype(np.float32))
    w_qkv = np.ascontiguousarray(w_qkv.astype(np.float32))
    w_proj = np.ascontiguousarray(w_proj.astype(np.float32))
    bp = np.ascontiguousarray(b_proj.reshape(1, C).astype(np.float32))
    return [
        {"x": xb[b], "w_qkv": w_qkv, "w_proj": w_proj, "b_proj": bp}
        for b in range(B)
    ]


def _run(in_maps, **kwargs):
    from concourse.bass_utils import run_bass_kernel_spmd

    nc = _get_nc()
    return run_bass_kernel_spmd(
        nc, in_maps, core_ids=list(range(NCORES)), **kwargs
    )


def kernel(x, w_qkv, w_proj, b_proj):
    B, H, W, _ = x.shape
    res = _run(_make_in_maps(x, w_qkv, w_proj, b_proj))
    out = np.stack([res.results[b]["y"] for b in range(B)])
    return out.reshape(B, H, W, C).astype(np.float32)
